# revision 6
# baseline (speedup 1.0000x reference)
"""Trainium2 Bass kernel for nn_MicroSpeech: 2-layer diagonal complex LRU net.

Math: |lam| = exp(-exp(nu)) ~= 0.368 for nu ~ U[0, 0.01), so the linear
recurrence h_t = lam*h_{t-1} + u_t decays by ~1e-7 within 16 steps. The scan is
therefore an exact-to-fp32 16-tap FIR, factorized as radix-(4,4):
    h_t = sum_{j=0..3} lam^{4j} * (sum_{k=0..3} lam^k u_{t-4j-k})
Each stage is a dense matmul over the stacked real/imag representation, with
taps pair-stacked along the 128-contraction dim. The C-projection absorbs the
stage-B taps of layer 1; selu is decomposed as
    selu(v) = L*relu(v) + L*A*(exp(min(v,0)) - 1)
with the affine pieces folded into downstream matmul weights / bias vectors.

Sharding: frames are split 8192/core across 8 cores; each core only needs a
30-frame halo of input (no inter-core communication). Each core runs 17 tiles
of 482 output frames (512-frame windows incl. halo; single PSUM bank per
stage).
"""
import os

os.environ.setdefault("MYCRO_LOCAL_CACHE", "1")

import numpy as np

WINDOW = 128
H = 32
O2 = 256
L_TOTAL = 65536
NCORES = 8
F = L_TOTAL // NCORES          # frames per core
HALO = 30
NIN = 482                      # interior frames per tile
NTILES = (F + NIN - 1) // NIN  # 17
# padded per-core input: frame p of the slice is global frame (core*F - HALO + p);
# tile i loads padded frames [NIN*i, NIN*i + 512)
PAD_FRAMES = NIN * (NTILES - 1) + 512  # 8224

SELU_L = 1.0507009873554805
SELU_A = 1.6732632423543772

# matmul dtype: "f32" (exact, 4 cy/row) | "f32r" (fast fp32, 1 cy/row) |
MM_DT = os.environ.get("MICROSPEECH_MM_DT", "f32r")


# ---------------------------------------------------------------- host precompute
def _build_consts(inp):
    def Trep(mu):
        a, b = np.diag(mu.real), np.diag(mu.imag)
        return np.block([[a, -b], [b, a]])

    def layer(br, bi, nu, th):
        br, bi, nu, th = [np.asarray(a, np.float64) for a in (br, bi, nu, th)]
        lam = np.exp(-np.exp(nu) + 1j * np.exp(th))
        gamma = np.sqrt(1.0 - np.abs(lam) ** 2)
        B = (br + 1j * bi) * gamma[:, None]
        return lam, B

    lam1, B1 = layer(inp["b1r"], inp["b1i"], inp["nu1"], inp["th1"])
    lam2, B2 = layer(inp["b2r"], inp["b2i"], inp["nu2"], inp["th2"])
    C1 = np.asarray(inp["c1r"], np.float64) + 1j * np.asarray(inp["c1i"], np.float64)
    C2 = np.asarray(inp["c2r"], np.float64) + 1j * np.asarray(inp["c2i"], np.float64)
    D1 = np.asarray(inp["d1"], np.float64)
    D2 = np.asarray(inp["d2"], np.float64)
    W = np.asarray(inp["mlp_w"], np.float64)
    b = np.asarray(inp["mlp_b"], np.float64)

    o = {}
    o["lhsT_u1"] = np.vstack([B1.real, B1.imag]).T                      # (128, 64)
    o["lhsT_A1_0"] = np.hstack([Trep(lam1 ** 0), Trep(lam1 ** 1)]).T    # (128, 64)
    o["lhsT_A1_1"] = np.hstack([Trep(lam1 ** 2), Trep(lam1 ** 3)]).T

    def Eproj(C, mu):
        Cr, Ci = C.real, C.imag
        return np.hstack([Cr * mu.real[None, :] - Ci * mu.imag[None, :],
                          -Cr * mu.imag[None, :] - Ci * mu.real[None, :]])

    E = [Eproj(C1, lam1 ** (4 * j)) for j in range(4)]
    o["lhsT_B1_0"] = np.hstack([E[0], E[1]]).T                          # (128, 32)
    o["lhsT_B1_1"] = np.hstack([E[2], E[3]]).T
    o["lhsT_D1"] = D1.T                                                 # (128, 32)

    o["lhsT_mlp"] = np.vstack([W, SELU_L * SELU_A * W])                 # (64, 32)
    beta = b - SELU_L * SELU_A * W.T @ np.ones(H)
    o["beta"] = beta                                                    # (32,)
    o["ls_beta"] = SELU_L * beta

    B2s = np.vstack([B2.real, B2.imag])                                 # (64, 32)
    o["lhsT_u2"] = np.hstack([B2s, SELU_L * SELU_A * B2s]).T            # (64, 64)
    u2_0 = B2s @ (-SELU_L * SELU_A * np.ones(H))
    o["lhsT_A2_0"] = np.hstack([Trep(lam2 ** 0), Trep(lam2 ** 1)]).T
    o["lhsT_A2_1"] = np.hstack([Trep(lam2 ** 2), Trep(lam2 ** 3)]).T
    Tsum_A = sum(Trep(lam2 ** k) for k in range(4))
    Tsum_B = sum(Trep(lam2 ** (4 * j)) for j in range(4))
    h2_0 = Tsum_B @ (Tsum_A @ u2_0)
    o["lhsT_B2_0"] = np.hstack([Trep(lam2 ** 0), Trep(lam2 ** 4)]).T    # (128, 64)
    o["lhsT_B2_1"] = np.hstack([Trep(lam2 ** 8), Trep(lam2 ** 12)]).T
    # proj stack order: [c2(0:32); e2(32:64); h2(64:128)]
    G = np.hstack([D2, SELU_L * SELU_A * D2, C2.real, -C2.imag])        # (256, 128)
    o["lhsT_P2a"] = G[:128].T                                           # (128, 128)
    o["lhsT_P2b"] = G[128:].T
    o["y2_0"] = (np.hstack([C2.real, -C2.imag]) @ h2_0
                 + D2 @ (-SELU_L * SELU_A * np.ones(H)))                # (256,)
    return {k: np.asarray(v) for k, v in o.items()}


# wts blob column layout (128 rows, f32)
_BLOB_SPECS = [
    ("ident", 128), ("lhsT_u1", 64), ("lhsT_A1_0", 64), ("lhsT_A1_1", 64),
    ("lhsT_B1_0", 32), ("lhsT_B1_1", 32), ("lhsT_D1", 32), ("lhsT_mlp", 32),
    ("lhsT_u2", 64), ("lhsT_A2_0", 64), ("lhsT_A2_1", 64),
    ("lhsT_B2_0", 64), ("lhsT_B2_1", 64), ("lhsT_P2a", 128), ("lhsT_P2b", 128),
    ("beta", 1), ("ls_beta", 1), ("y2_0a", 1), ("y2_0b", 1),
]
_BLOB_OFF = {}
_c = 0
for _n, _w in _BLOB_SPECS:
    _BLOB_OFF[_n] = _c
    _c += _w
BLOB_COLS = _c


def _pack_blob(consts):
    blob = np.zeros((128, BLOB_COLS), np.float32)
    blob[:, :128] = np.eye(128, dtype=np.float32)
    for name, wdt in _BLOB_SPECS:
        if name == "ident":
            continue
        off = _BLOB_OFF[name]
        if name == "beta":
            blob[:H, off] = consts["beta"]
        elif name == "ls_beta":
            blob[:H, off] = consts["ls_beta"]
        elif name == "y2_0a":
            blob[:, off] = consts["y2_0"][:128]
        elif name == "y2_0b":
            blob[:, off] = consts["y2_0"][128:]
        else:
            m = consts[name].astype(np.float32)
            blob[: m.shape[0], off: off + m.shape[1]] = m
    return blob


# ---------------------------------------------------------------- bass program
_PROGRAM = None


def _build_program():
    import concourse.bacc as bacc
    import concourse.tile as tile
    from concourse import mybir

    nc = bacc.Bacc(None, target_bir_lowering=False)
    dt = mybir.dt
    AF = mybir.ActivationFunctionType
    ALU = mybir.AluOpType

    xin = nc.declare_dram_parameter("xin", [PAD_FRAMES, WINDOW], dt.float32, isOutput=False)
    wts_d = nc.declare_dram_parameter("wts", [128, BLOB_COLS], dt.float32, isOutput=False)
    yout = nc.declare_dram_parameter("yout", [O2, F], dt.float32, isOutput=True)

    mmdt = {"f32": dt.float32, "f32r": dt.float32r}[MM_DT]

    def W(name, p=128):
        """lhsT AP from the weights sbuf blob."""
        off = _BLOB_OFF[name]
        wdt = dict(_BLOB_SPECS)[name]
        return wts[:p, off: off + wdt]

    with tile.TileContext(nc) as tc:
        with (
            tc.tile_pool(name="singles", bufs=1) as singles,
            tc.tile_pool(name="work", bufs=2) as work,
            tc.tile_pool(name="psum", bufs=8, space="PSUM") as psum,
        ):
            wts = singles.tile([128, BLOB_COLS], dt.float32)
            nc.sync.dma_start(out=wts, in_=wts_d[:, :])

            def mm(out, lhsT, rhs, start, stop):
                nc.tensor.matmul(out, lhsT.bitcast(mmdt), rhs.bitcast(mmdt),
                                 start=start, stop=stop)

            for i in range(NTILES):
                nint = min(NIN, F - NIN * i)
                f0 = NIN * i
                c0 = NIN * i  # output col base

                # -------- load + transpose x window: (128 w, 512 frames)
                s4 = work.tile([128, 512], dt.float32, tag="s4")
                nc.sync.dma_start(
                    out=s4[:, :].rearrange("p (b w) -> p b w", b=4),
                    in_=xin[f0: f0 + 512, :].rearrange("(b p) w -> p b w", p=128))
                xT = psum.tile([128, 512], dt.float32, tag="ps")
                for bb in range(4):
                    nc.tensor.transpose(
                        xT[:, bb * 128:(bb + 1) * 128],
                        s4[:, bb * 128:(bb + 1) * 128], wts[:, 0:128])
                x = work.tile([128, 512], dt.float32, tag="x")
                nc.vector.tensor_copy(out=x, in_=xT)

                # -------- layer 1: u1 = B~1 @ x  (frames [0,512))
                u1ps = psum.tile([64, 512], dt.float32, tag="ps")
                mm(u1ps, W("lhsT_u1"), x, True, True)
                U1 = work.tile([128, 512], dt.float32, tag="U1")
                nc.vector.tensor_copy(out=U1[0:64, :], in_=u1ps)
                nc.gpsimd.tensor_copy(out=U1[64:128, 1:512], in_=U1[0:64, 0:511])

                # stage A1 -> p1 frames [3,512)
                p1ps = psum.tile([64, 509], dt.float32, tag="ps")
                mm(p1ps, W("lhsT_A1_0"), U1[:, 3:512], True, False)
                mm(p1ps, W("lhsT_A1_1"), U1[:, 1:510], False, True)
                P1 = work.tile([128, 512], dt.float32, tag="P1")
                nc.vector.tensor_copy(out=P1[0:64, 3:512], in_=p1ps)
                nc.gpsimd.tensor_copy(out=P1[64:128, 7:512], in_=P1[0:64, 3:508])

                # stage B1 + C1 proj + D1 -> y1 frames [15,512)
                y1ps = psum.tile([32, 497], dt.float32, tag="ps")
                mm(y1ps, W("lhsT_B1_0"), P1[:, 15:512], True, False)
                mm(y1ps, W("lhsT_B1_1"), P1[:, 7:504], False, False)
                mm(y1ps, W("lhsT_D1"), x[:, 15:512], False, True)

                # selu1 -> CE1 = [c1; e1]
                CE1 = work.tile([64, 512], dt.float32, tag="CE1")
                nc.scalar.activation(out=CE1[0:32, 15:512], in_=y1ps,
                                     func=AF.Relu, scale=SELU_L)
                m1 = work.tile([32, 512], dt.float32, tag="m1")
                nc.vector.tensor_scalar_min(out=m1[:, 15:512], in0=y1ps, scalar1=0.0)
                nc.scalar.activation(out=CE1[32:64, 15:512], in_=m1[:, 15:512],
                                     func=AF.Exp)

                # mlp -> z frames [15,512)
                zps = psum.tile([32, 497], dt.float32, tag="ps")
                mm(zps, W("lhsT_mlp", p=64), CE1[:, 15:512], True, True)

                # selu2 -> Z2 = [c2; e2; h2]
                Z2 = work.tile([128, 512], dt.float32, tag="Z2")
                nc.scalar.activation(out=Z2[0:32, 15:512], in_=zps,
                                     func=AF.Relu, scale=SELU_L,
                                     bias=wts[0:32, _BLOB_OFF["ls_beta"]:_BLOB_OFF["ls_beta"] + 1])
                m2 = work.tile([32, 512], dt.float32, tag="m2")
                nc.vector.tensor_scalar(
                    out=m2[:, 15:512], in0=zps,
                    scalar1=wts[0:32, _BLOB_OFF["beta"]:_BLOB_OFF["beta"] + 1],
                    scalar2=0.0, op0=ALU.add, op1=ALU.min)
                nc.scalar.activation(out=Z2[32:64, 15:512], in_=m2[:, 15:512],
                                     func=AF.Exp)

                # layer 2: u2 frames [15,512)
                u2ps = psum.tile([64, 497], dt.float32, tag="ps")
                mm(u2ps, W("lhsT_u2", p=64), Z2[0:64, 15:512], True, True)
                U2 = work.tile([128, 512], dt.float32, tag="U2")
                nc.vector.tensor_copy(out=U2[0:64, 15:512], in_=u2ps)
                nc.gpsimd.tensor_copy(out=U2[64:128, 16:512], in_=U2[0:64, 15:511])

                # stage A2 -> p2 frames [18,512)
                p2ps = psum.tile([64, 494], dt.float32, tag="ps")
                mm(p2ps, W("lhsT_A2_0"), U2[:, 18:512], True, False)
                mm(p2ps, W("lhsT_A2_1"), U2[:, 16:510], False, True)
                P2 = work.tile([128, 512], dt.float32, tag="P2")
                nc.vector.tensor_copy(out=P2[0:64, 18:512], in_=p2ps)
                nc.gpsimd.tensor_copy(out=P2[64:128, 22:512], in_=P2[0:64, 18:508])

                # stage B2 -> h2 frames [30, 30+nint)
                h2ps = psum.tile([64, 482], dt.float32, tag="ps")
                mm(h2ps[:, :nint], W("lhsT_B2_0"), P2[:, 30:30 + nint], True, False)
                mm(h2ps[:, :nint], W("lhsT_B2_1"), P2[:, 22:22 + nint], False, True)
                nc.vector.tensor_copy(out=Z2[64:128, 30:30 + nint], in_=h2ps[:, :nint])

                # projection -> y2 (256 = 2x128)
                for half, ytag in ((0, "ya"), (1, "yb")):
                    yps = psum.tile([128, 482], dt.float32, tag="ps")
                    mm(yps[:, :nint], W("lhsT_P2a" if half == 0 else "lhsT_P2b"),
                       Z2[:, 30:30 + nint], True, True)
                    yo = work.tile([128, 482], dt.float32, tag=ytag)
                    bcol = _BLOB_OFF["y2_0a" if half == 0 else "y2_0b"]
                    nc.vector.tensor_scalar_add(
                        out=yo[:, :nint], in0=yps[:, :nint],
                        scalar1=wts[:, bcol:bcol + 1])
                    nc.sync.dma_start(
                        out=yout[half * 128:(half + 1) * 128, c0:c0 + nint],
                        in_=yo[:, :nint])
    nc.finalize()
    return nc


def _get_program():
    global _PROGRAM
    if _PROGRAM is None:
        _PROGRAM = _build_program()
    return _PROGRAM


# ---------------------------------------------------------------- host wrapper
def _make_inmaps(inputs):
    consts = _build_consts(inputs)
    blob = _pack_blob(consts)
    ts = np.asarray(inputs["inputs_timeseries"], np.float32).ravel()
    in_maps = []
    for core in range(NCORES):
        s0 = core * F
        xpad = np.zeros((PAD_FRAMES * WINDOW,), np.float32)
        g0 = (s0 - HALO) * WINDOW
        g1 = min((s0 + PAD_FRAMES - HALO) * WINDOW, ts.size)
        a0 = max(0, -g0)
        xpad[a0: a0 + (g1 - max(g0, 0))] = ts[max(g0, 0): g1]
        in_maps.append({"xin": xpad.reshape(PAD_FRAMES, WINDOW), "wts": blob})
    return in_maps


def _enable_axon_trace():
    """Shim the missing antenv.axon_hooks so trace=True works under axon."""
    import sys
    import types

    if "antenv.axon_hooks" not in sys.modules:
        from trn_agent_boot.trn_boot import _ntff_profile_via_ctypes

        mod = types.ModuleType("antenv.axon_hooks")
        state = {"hook": None}
        mod.set_axon_ntff_profile_hook = lambda h: state.__setitem__("hook", h)
        mod.get_axon_ntff_profile_hook = lambda: state["hook"]
        sys.modules["antenv.axon_hooks"] = mod
        try:
            import antenv

            antenv.axon_hooks = mod
        except ImportError:
            pass
        hook = _ntff_profile_via_ctypes("/opt/axon/libaxon_pjrt.so")
        assert hook is not None
        mod.set_axon_ntff_profile_hook(hook)
    # keep trace artifacts local (no bucket access in this container)
    import concourse.bass_utils as bu

    bu.upload_artifacts = lambda tmpdir: tmpdir


def run(inputs, trace=False, **trace_kwargs):
    from concourse.bass_utils import run_bass_kernel_spmd

    if trace:
        _enable_axon_trace()
    nc = _get_program()
    in_maps = _make_inmaps(inputs)
    res = run_bass_kernel_spmd(nc, in_maps, list(range(NCORES)), trace=trace,
                               **trace_kwargs)
    out = np.concatenate([r["yout"] for r in res.results], axis=1)
    return out.astype(np.float32), res


def kernel(**inputs) -> np.ndarray:
    out, _ = run(inputs)
    return out
